# revision 21
# baseline (speedup 1.0000x reference)
"""Trainium2 Bass kernel for nn_EncoderLayer_73315091743398.

The reference attention einsums ('hwink,hwijm->hwinm') sum their k/j indices
independently, so per (h,w)-chunk c and head i, over the flat matrix
xf = x viewed (8192, 512) in raw (s,h,w) row order with qf = xf@Wq.T + pe:

    u[s]  = sum_{d in seg_i} qf[c*512+s, d]        (segment row sums)
    a     = softmax_s(u)
    v[d]  = sum_s a[s] * qf[c*512+s, 64i+d]
    row   = tile8(v) @ Wfc.T = v @ M,   M[d,:] = sum_b Wfc[:, 64b+d].T

and attn_out viewed (S,H,W,D) has row A[s'] = row_{c=s'//32, i=(s'%32)//4}.
q = x@Wq.T is never materialized:
    u = X_c @ wseg + pu          (wseg = per-head column sums of Wq.T)
    YT = X.T @ exT ; GT = WqT.T @ YT + peC.T @ apeT ; v_i = GT[seg_i, i]
(softmax normalization folded into one scale on las).  Both residual adds
ride the PE as identity matmuls accumulating into PSUM, so each LayerNorm
reads its z directly from PSUM.

Biases fold away: g1 into W1 (host), be1/b1 into the relu bias (host),
b2/be1 vanish from the second LayerNorm by shift invariance.

Core k owns flat rows [k*1024,(k+1)*1024) = attention chunks {2k, 2k+1};
8 cores run fully independent SPMD programs (no collectives).
"""

import math
import os
import sys
from contextlib import ExitStack

import numpy as np
import ml_dtypes  # noqa: F401  (registers bfloat16)

for _p in ("/opt/trn_rl_repo", "/root/.axon_site/_ro/trn_rl_repo"):
    if os.path.isdir(_p) and _p not in sys.path:
        sys.path.append(_p)

import concourse.bass as bass  # noqa: F401
import concourse.bacc as bacc
import concourse.mybir as mybir
import concourse.tile as tile
from concourse.bass_utils import run_bass_kernel_spmd

F32 = mybir.dt.float32
F32R = mybir.dt.float32r
FP16 = mybir.dt.float16
AF = mybir.ActivationFunctionType
ALU = mybir.AluOpType
AX = mybir.AxisListType

S, H, W, D = 512, 4, 4, 512
NH, DEP, DFF = 8, 64, 2048
NCORES = 8
R = 1024          # rows per core of the flat (8192, 512) view
EPS = 1e-5

# PKH (fp16, [128, 1152]): eyeT [0:128], Mst [128:640], E8 at rows 0:8 of
# [640:1152]
PKH_EYE, PKH_MST, PKH_E8, PKH_N = 0, 128, 640, 1152
# PK32 (f32r, [32, 528]): REP [0:512], puT [512:528]
PK32_REP, PK32_PUT, PK32_N = 0, 512, 528

_cached = {}


def build_nc(flags):
    """Build the single-core SPMD Bass/Tile program (same program on all 8)."""
    need_g1, need_g2, need_be2 = flags
    nc = bacc.Bacc("TRN2", debug=False, target_bir_lowering=False)

    d_PKH = nc.dram_tensor("PKH", [128, PKH_N], FP16, kind="ExternalInput")
    d_wsegT = nc.dram_tensor("wsegT", [128, 32], FP16, kind="ExternalInput")
    d_PK32 = nc.dram_tensor("PK32", [32, PK32_N], F32R, kind="ExternalInput")
    d_eyeS = nc.dram_tensor("eyeS", [8, 8], FP16, kind="ExternalInput")
    d_peC = nc.dram_tensor("peC", [32, 1024], FP16, kind="ExternalInput")
    d_b1g = nc.dram_tensor("b1g", [128, 16], F32, kind="ExternalInput")
    d_xT = [nc.dram_tensor(f"xT{c}", [128, 2048], FP16, kind="ExternalInput")
            for c in range(2)]
    d_xR = [nc.dram_tensor(f"xR{c}", [128, 2048], FP16, kind="ExternalInput")
            for c in range(2)]
    d_wq = nc.dram_tensor("wqT", [128, 2048], FP16, kind="ExternalInput")
    d_w1 = nc.dram_tensor("w1t", [128, 8192], FP16, kind="ExternalInput")
    d_w2 = nc.dram_tensor("w2t", [128, 8192], FP16, kind="ExternalInput")
    if need_g1:
        d_g1 = nc.dram_tensor("g1R", [128, 512], F32, kind="ExternalInput")
    if need_g2:
        d_g2 = nc.dram_tensor("g2R", [128, 512], F32, kind="ExternalInput")
    if need_be2:
        d_be2 = nc.dram_tensor("be2R", [128, 512], F32, kind="ExternalInput")
    d_out = nc.dram_tensor("out", [R, D], FP16, kind="ExternalOutput")

    with ExitStack() as ctx:
        tc = ctx.enter_context(tile.TileContext(nc))
        cst = ctx.enter_context(tc.tile_pool(name="cst", bufs=1))
        xp = ctx.enter_context(tc.tile_pool(name="xp", bufs=1))
        qp = ctx.enter_context(tc.tile_pool(name="qp", bufs=1))
        wk = ctx.enter_context(tc.tile_pool(name="wk", bufs=2))
        ps = ctx.enter_context(tc.tile_pool(name="ps", bufs=1, space="PSUM"))

        # ---- loads: ONE queue (sync), exact need order; stores on gpsimd ----
        xTs = [xp.tile([128, 2048], FP16, tag=f"xT{c}", name=f"xT{c}")
               for c in range(2)]
        xRs = [xp.tile([128, 2048], FP16, tag=f"xR{c}", name=f"xR{c}")
               for c in range(2)]
        eyeS = cst.tile([8, 8], FP16, tag="eyeS", name="eyeS")
        nc.sync.dma_start(eyeS[:], d_eyeS[:])
        wsegT = cst.tile([128, 32], FP16, tag="wsegT", name="wsegT")
        nc.sync.dma_start(wsegT[:], d_wsegT[:])
        nc.sync.dma_start(xTs[0][:], d_xT[0][:])
        PK32 = cst.tile([32, PK32_N], F32R, tag="PK32", name="PK32")
        nc.sync.dma_start(PK32[:], d_PK32[:])
        nc.sync.dma_start(xRs[0][:], d_xR[0][:])
        wqs = cst.tile([128, 2048], FP16, tag="wqs", name="wqs")
        nc.sync.dma_start(wqs[:], d_wq[:])
        peC = cst.tile([32, 1024], FP16, tag="peC", name="peC")
        nc.sync.dma_start(peC[:], d_peC[:])
        PKH = cst.tile([128, PKH_N], FP16, tag="PKH", name="PKH")
        nc.sync.dma_start(PKH[:], d_PKH[:])
        nc.sync.dma_start(xTs[1][:], d_xT[1][:])
        nc.sync.dma_start(xRs[1][:], d_xR[1][:])
        b1g = cst.tile([128, 16], F32, tag="b1g", name="b1g")
        nc.sync.dma_start(b1g[:], d_b1g[:])
        w1s = cst.tile([128, 8192], FP16, tag="w1s", name="w1s")
        nc.sync.dma_start(w1s[:], d_w1[:])
        w2s = cst.tile([128, 8192], FP16, tag="w2s", name="w2s")
        nc.sync.dma_start(w2s[:], d_w2[:])
        if need_g1:
            g1R = cst.tile([128, 512], F32, tag="g1R", name="g1R")
            nc.sync.dma_start(g1R[:], d_g1[:])
        if need_g2:
            g2R = cst.tile([128, 512], F32, tag="g2R", name="g2R")
            nc.sync.dma_start(g2R[:], d_g2[:])
        if need_be2:
            be2R = cst.tile([128, 512], F32, tag="be2R", name="be2R")
            nc.sync.dma_start(be2R[:], d_be2[:])

        eyeT = PKH[:, PKH_EYE:PKH_EYE + 128]
        Mst = PKH[:, PKH_MST:PKH_MST + 512]
        E8 = PKH[0:8, PKH_E8:PKH_E8 + 512]
        REP = PK32[:, PK32_REP:PK32_REP + 512]
        puT = PK32[:, PK32_PUT:PK32_PUT + 16]

        epsT = cst.tile([128, 1], F32, tag="eps", name="epsT")
        nc.vector.memset(epsT[:], EPS)
        zer8 = cst.tile([128, 8], F32, tag="zer8", name="zer8")
        nc.vector.memset(zer8[:], 0.0)

        # persistent per-core activations
        o1T = [xp.tile([128, 4 * 512], FP16, tag=f"o1T{c}", name=f"o1T{c}")
               for c in range(2)]
        h1 = [xp.tile([128, 16 * 512], FP16, tag=f"h1_{c}", name=f"h1_{c}")
              for c in range(2)]
        nrm1 = [qp.tile([128, 512], FP16, tag=f"nrm{m}", name=f"nrm{m}")
                for m in range(8)]
        lassb = [qp.tile([8, 512], FP16, tag=f"las{c}", name=f"las{c}")
                 for c in range(2)]

        # ------------- attention (both chunks, station-interleaved) ---------
        ups, exs, rcps, apes, exTs = {}, {}, {}, {}, {}

        def u_stage(c):
            u = ps.tile([8, 512], F32, tag="sm", bufs=2)
            for jb in range(4):
                nc.tensor.matmul(u[:], wsegT[:, jb * 8:(jb + 1) * 8],
                                 xTs[c][:, jb * 512:(jb + 1) * 512],
                                 start=(jb == 0), stop=False)
            nc.tensor.matmul(u[:], puT[:, c * 8:(c + 1) * 8], REP,
                             start=False, stop=True)
            ups[c] = u

        def softmax_a(c):
            mx = wk.tile([8, 1], F32, tag="mx")
            nc.vector.tensor_reduce(mx[:], ups[c][:], axis=AX.X, op=ALU.max)
            nmx = wk.tile([8, 1], F32, tag="nmx")
            nc.vector.tensor_scalar_mul(nmx[:], mx[:], -1.0)
            ex = wk.tile([8, 512], FP16, tag=f"ex{c}", bufs=1)
            ssum = wk.tile([8, 1], F32, tag=f"esum{c}", bufs=1)
            nc.scalar.activation(ex[:], ups[c][:], AF.Exp, bias=nmx[:, :],
                                 accum_out=ssum[:])
            exs[c] = ex
            return ssum

        def exT_stage(c):
            atp = ps.tile([128, 32], FP16, tag="tp", bufs=2)
            for sb in range(4):
                nc.tensor.transpose(atp[:, sb * 8:(sb + 1) * 8],
                                    exs[c][:, sb * 128:(sb + 1) * 128],
                                    eyeS[:])
            t = wk.tile([128, 32], FP16, tag=f"aT{c}", bufs=1)
            nc.vector.tensor_copy(t[:], atp[:])
            exTs[c] = t

        def post_stage(c, ssum):
            rcp = wk.tile([8, 1], F32, tag=f"rcp{c}", bufs=1)
            nc.vector.reciprocal(rcp[:], ssum[:])
            rcps[c] = rcp
            ape = wk.tile([8, 32], FP16, tag=f"ape{c}", bufs=1)
            with nc.allow_low_precision(reason="ape: 16-term sums of ex<=1"):
                nc.vector.tensor_reduce(
                    ape[:], exs[c][:].rearrange("p (t u) -> p t u", t=32),
                    axis=AX.X, op=ALU.add)
            aptp = ps.tile([32, 8], FP16, tag="tp", bufs=2)
            nc.tensor.transpose(aptp[:], ape[:], eyeS[:])
            apeT = wk.tile([32, 8], FP16, tag=f"apeT{c}", bufs=1)
            nc.vector.tensor_copy(apeT[:], aptp[:])
            apes[c] = apeT

        def yt_stage(c):
            # YT[j, i] = sum_s X[s, j] * ex[i, s], per j-block
            eT = exTs[c]
            ytp = ps.tile([128, 32], F32, tag="tp", bufs=2)
            for jb in range(4):
                for sb in range(4):
                    nc.tensor.matmul(
                        ytp[:, jb * 8:(jb + 1) * 8],
                        xRs[c][:, sb * 512 + jb * 128:sb * 512 + (jb + 1) * 128],
                        eT[:, sb * 8:(sb + 1) * 8],
                        start=(sb == 0), stop=(sb == 3))
            t = wk.tile([128, 32], FP16, tag=f"yT{c}", bufs=1)
            nc.vector.tensor_copy(t[:], ytp[:])
            return t

        def gt_las_stage(c, yT):
            # GT[e, i] = sum_j WqT[j, e]*YT[j, i] + sum_t peC[t, e]*apeT[t, i]
            apeT = apes[c]
            vm = wk.tile([128, 8], FP16, tag=f"vm{c}", bufs=1)
            nc.vector.tensor_copy(vm[:], zer8[:])
            gtp = ps.tile([128, 32], F32, tag="tp", bufs=2)
            for eb in range(4):
                for jb in range(4):
                    nc.tensor.matmul(
                        gtp[:, eb * 8:(eb + 1) * 8],
                        wqs[:, jb * 512 + eb * 128:jb * 512 + (eb + 1) * 128],
                        yT[:, jb * 8:(jb + 1) * 8], start=(jb == 0), stop=False)
                nc.tensor.matmul(
                    gtp[:, eb * 8:(eb + 1) * 8],
                    peC[:, c * 512 + eb * 128:c * 512 + (eb + 1) * 128],
                    apeT[:], start=False, stop=True)
            for eb in range(4):
                nc.vector.tensor_copy(
                    vm[0:64, 2 * eb:2 * eb + 1],
                    gtp[0:64, eb * 8 + 2 * eb:eb * 8 + 2 * eb + 1])
                nc.vector.tensor_copy(
                    vm[64:128, 2 * eb + 1:2 * eb + 2],
                    gtp[64:128, eb * 8 + 2 * eb + 1:eb * 8 + 2 * eb + 2])
            lps = ps.tile([8, 512], F32, tag="sm", bufs=2)
            nc.tensor.matmul(lps[:], vm[:], Mst, start=True, stop=True)
            nc.vector.tensor_scalar_mul(lassb[c][:], lps[:], rcps[c][:])

        def resid_ln1(c, jt):
            """z1 = broadcast(las) + x (both on PE); nrm1 = LN(z1), m=c*4+jt."""
            m = c * 4 + jt
            bcp = ps.tile([128, 512], F32, tag="mm", bufs=2)
            nc.tensor.matmul(bcp[:], E8[:, jt * 128:(jt + 1) * 128],
                             lassb[c][:], start=True, stop=False)
            nc.tensor.matmul(bcp[:], eyeT,
                             xRs[c][:, jt * 512:(jt + 1) * 512],
                             start=False, stop=True)
            st6 = wk.tile([128, 6], F32, tag="ls")
            nc.vector.bn_stats(st6[:], bcp[:])
            mv = wk.tile([128, 2], F32, tag="lm")
            nc.vector.bn_aggr(mv[:], st6[:])
            sd = wk.tile([128, 1], F32, tag="lsd")
            nc.scalar.activation(sd[:], mv[:, 1:2], AF.Sqrt, bias=epsT[:, :])
            rsd = wk.tile([128, 1], F32, tag="lr")
            nc.vector.reciprocal(rsd[:], sd[:])
            nmr = wk.tile([128, 1], F32, tag="nmr")
            nc.vector.tensor_scalar(nmr[:], mv[:, 0:1], rsd[:], -1.0,
                                    op0=ALU.mult, op1=ALU.mult)
            nc.scalar.activation(nrm1[m][:], bcp[:], AF.Identity,
                                 bias=nmr[:, :], scale=rsd[:, :])
            if need_g1:
                nc.vector.tensor_mul(nrm1[m][:], nrm1[m][:], g1R[:])

        def trans_stage(m):
            c, jt = divmod(m, 4)
            tr = ps.tile([128, 512], FP16, tag="sm", bufs=2)
            for dt in range(4):
                nc.tensor.transpose(tr[:, dt * 128:(dt + 1) * 128],
                                    nrm1[m][:, dt * 128:(dt + 1) * 128],
                                    eyeT)
            dst = (o1T[c][:].rearrange("p (dt s) -> p dt s", dt=4)
                   [:, :, jt * 128:(jt + 1) * 128])
            nc.scalar.copy(dst, tr[:].rearrange("p (dt s) -> p dt s", dt=4))

        # ---------------- FFN ----------------
        def mm1(c, ft):
            hps = ps.tile([128, 512], F32, tag="mmh", bufs=2)
            for dt in range(4):
                nc.tensor.matmul(
                    hps[:],
                    w1s[:, dt * 2048 + ft * 128:dt * 2048 + (ft + 1) * 128],
                    o1T[c][:, dt * 512:(dt + 1) * 512],
                    start=(dt == 0), stop=(dt == 3))
            nc.scalar.activation(h1[c][:, ft * 512:(ft + 1) * 512], hps[:],
                                 AF.Relu, bias=b1g[:, ft:ft + 1])

        def mm2_ln2(c, sb):
            """s-block c*4+sb: z2 = h1.T @ W2.T + nrm1 in PSUM; out = LN(z2)."""
            m = c * 4 + sb
            p2 = ps.tile([128, 512], F32, tag="mm", bufs=2)
            for ft in range(16):
                nc.tensor.matmul(
                    p2[:],
                    h1[c][:, ft * 512 + sb * 128:ft * 512 + (sb + 1) * 128],
                    w2s[:, ft * 512:(ft + 1) * 512],
                    start=(ft == 0), stop=(ft == 15))
            z2 = wk.tile([128, 512], F32, tag="z2", bufs=2)
            nc.vector.tensor_add(z2[:], p2[:], nrm1[m][:])
            st6 = wk.tile([128, 6], F32, tag="ls2")
            nc.vector.bn_stats(st6[:], z2[:])
            mv = wk.tile([128, 2], F32, tag="lm2")
            nc.vector.bn_aggr(mv[:], st6[:])
            sd = wk.tile([128, 1], F32, tag="lsd2")
            nc.scalar.activation(sd[:], mv[:, 1:2], AF.Sqrt, bias=epsT[:, :])
            rsd = wk.tile([128, 1], F32, tag="lr2")
            nc.vector.reciprocal(rsd[:], sd[:])
            yt = wk.tile([128, 512], FP16, tag="yt", bufs=2)
            nc.vector.tensor_scalar(yt[:], z2[:], mv[:, 0:1], rsd[:],
                                    op0=ALU.subtract, op1=ALU.mult)
            if need_g2:
                nc.vector.tensor_mul(yt[:], yt[:], g2R[:])
            if need_be2:
                nc.vector.tensor_add(yt[:], yt[:], be2R[:])
            nc.gpsimd.dma_start(d_out[m * 128:(m + 1) * 128, :], yt[:])

        # ---------------- schedule ----------------
        u_stage(0)
        u_stage(1)
        ss0 = softmax_a(0)
        ss1 = softmax_a(1)
        exT_stage(0)
        exT_stage(1)
        yt0 = yt_stage(0)
        yt1 = yt_stage(1)
        post_stage(0, ss0)
        post_stage(1, ss1)
        gt_las_stage(0, yt0)
        gt_las_stage(1, yt1)
        for jt in range(4):
            resid_ln1(0, jt)
        for m in range(4):
            trans_stage(m)
        for jt in range(4):
            resid_ln1(1, jt)
        for ft in range(16):
            mm1(0, ft)
        for m in range(4, 8):
            trans_stage(m)
        for sb in range(4):
            mm2_ln2(0, sb)
        for ft in range(16):
            mm1(1, ft)
        for sb in range(4):
            mm2_ln2(1, sb)

    nc.compile()
    return nc


def _round_f32r(a):
    b = np.ascontiguousarray(a, dtype=np.float32).view(np.uint32)
    out = (b + 0x7FF + ((b >> 12) & 1)) & np.uint32(0xFFFFF000)
    return out.view(np.float32)


def _pe_table():
    pos = np.arange(S, dtype=np.float32)[:, None]
    div = np.exp(np.arange(0, D, 2, dtype=np.float32) * (-math.log(10000.0) / D))
    ang = pos * div
    pe = np.zeros((S, D), np.float32)
    pe[:, 0::2] = np.sin(ang)
    pe[:, 1::2] = np.cos(ang)
    return pe


def _blk128(a):
    """[128*B, N] -> [128, B*N]: column block b holds rows [b*128,(b+1)*128)."""
    B = a.shape[0] // 128
    return np.ascontiguousarray(
        a.reshape(B, 128, a.shape[1]).transpose(1, 0, 2).reshape(128, -1))


def make_in_maps(x, Wq, Wfc, W1, b1, W2, b2, g1, be1, g2, be2):
    f32 = lambda a: np.ascontiguousarray(a, dtype=np.float32)
    fp16 = lambda a: np.ascontiguousarray(np.asarray(a, dtype=np.float32),
                                          ).astype(np.float16)
    x, Wq, Wfc, W1, W2 = f32(x), f32(Wq), f32(Wfc), f32(W1), f32(W2)
    b1, b2, g1, be1, g2, be2 = map(f32, (b1, b2, g1, be1, g2, be2))
    xf = x.reshape(S * H * W, D)
    pe = _pe_table()
    pe_seg = pe.reshape(S, NH, DEP).sum(-1)              # [s, i]
    wseg = Wq.reshape(NH, DEP, D).sum(1)                 # [i, j]
    M = Wfc.reshape(D, NH, DEP).sum(1).T                 # [64, 512]
    b1t = b1 + W1 @ be1

    need_g1 = bool(np.any(g1 != 1.0))
    need_g2 = bool(np.any(g2 != 1.0))
    need_be2 = bool(np.any(be2 != 0.0))
    flags = (need_g1, need_g2, need_be2)
    W1g = W1 if need_g1 else W1 * g1[None, :]

    PKH = np.zeros((128, PKH_N), np.float32)
    PKH[:, PKH_EYE:PKH_EYE + 128] = np.eye(128, dtype=np.float32)
    PKH[:, PKH_MST:PKH_MST + 512] = np.concatenate([M, M], axis=0)
    for jt in range(4):
        PKH[2 * jt, PKH_E8 + jt * 128 + np.arange(64)] = 1.0
        PKH[2 * jt + 1, PKH_E8 + jt * 128 + 64 + np.arange(64)] = 1.0
    PK32 = np.zeros((32, PK32_N), np.float32)
    PK32[:, PK32_REP:PK32_REP + 512] = (
        np.arange(512)[None, :] // 16 == np.arange(32)[:, None])

    shared = {
        "PKH": fp16(PKH),
        "wsegT": fp16(_blk128(wseg.T)),
        "eyeS": np.eye(8, dtype=np.float16),
        "b1g": b1t.reshape(16, 128).T.copy(),
        "wqT": fp16(_blk128(Wq.T)),                      # [p, jb*512+e]
        "w2t": fp16(_blk128(W2.T)),                      # [p, ft*512+e]
        "w1t": fp16(_blk128(W1g.T)),                     # [p, dt*2048+ff]
    }
    if need_g1:
        shared["g1R"] = np.tile(g1, (128, 1))
    if need_g2:
        shared["g2R"] = np.tile(g2, (128, 1))
    if need_be2:
        shared["be2R"] = np.tile(be2, (128, 1))

    maps = []
    for k in range(NCORES):
        sl = xf[k * R:(k + 1) * R]
        m = dict(shared)
        for c in range(2):
            ch = sl[c * 512:(c + 1) * 512]               # [s, j]
            m[f"xT{c}"] = fp16(_blk128(ch.T))            # [p, jb*512+s]
            m[f"xR{c}"] = fp16(_blk128(ch))              # [p, sb*512+j]
        crow = 2 * k * 32
        pk32 = PK32.copy()
        pk32[:, PK32_PUT:PK32_PUT + 16] = (
            pe_seg[crow:crow + 64].reshape(2, 32, NH).transpose(1, 0, 2)
            .reshape(32, 16))
        m["PK32"] = _round_f32r(pk32)
        m["peC"] = fp16(
            pe[crow:crow + 64].reshape(2, 32, D).transpose(1, 0, 2)
            .reshape(32, 1024))
        maps.append(m)
    return maps, flags


def kernel(x, Wq, Wfc, W1, b1, W2, b2, g1, be1, g2, be2, _results_hook=None,
           _trace=False, _tmpdir=None):
    in_maps, flags = make_in_maps(x, Wq, Wfc, W1, b1, W2, b2, g1, be1, g2, be2)
    if flags not in _cached:
        _cached[flags] = build_nc(flags)
    nc = _cached[flags]
    res = run_bass_kernel_spmd(nc, in_maps, list(range(NCORES)),
                               trace=_trace, tmpdir=_tmpdir)
    if _results_hook is not None:
        _results_hook(res)
    y = np.concatenate([np.asarray(res.results[k]["out"], dtype=np.float32)
                        for k in range(NCORES)], axis=0)
    return y.reshape(S, H, W, D)
